# revision 23
# baseline (speedup 1.0000x reference)
"""Trainium2 Bass kernel for a dense transformer block (nn_Block_120259084502).

Contract: kernel(**inputs) takes the FULL unsharded inputs (numpy, fp32) and
returns the FULL output [4, 2048, 1024] fp32. Internally shards across 8
NeuronCores: core c handles batch c//2, query-token half c%2. Each core
receives its batch's full 2048 tokens (rolled so its own 1024 query tokens
come first) and computes K/V for all of them locally, so no collectives are
needed (attention context = full batch; softmax is order-invariant so the
roll is harmless).

Precision strategy (validated against the reference on the host):
- The attention path runs in fp8 e4m3 with DoubleRow matmuls (2 rows/cycle):
  LN1 output h, Wq/Wk/Wv, V, the softmax probabilities P, cat and proj_w.
  P is written as exp(score*c - 8 + ln 128) directly in e4m3 by the scalar
  engine; the constant shift cancels exactly in the softmax normalization.
- The MLP runs the first NF8 f-chunks in fp8-DoubleRow and the rest in bf16;
  bf16 dw/hid are pre-scaled by the same power-of-two factors so both
  accumulate into one PSUM group with a single final dequant.
All scales are powers of two (exact), weight scales picked from host absmax.
LayerNorm statistics and both residual adds stay fp32.
"""

import math

import numpy as np
import ml_dtypes

import concourse.bacc as bacc
import concourse.tile as tile
from concourse import mybir
from concourse.bass_utils import run_bass_kernel_spmd
from concourse.masks import make_identity

bf16 = mybir.dt.bfloat16
fp8 = mybir.dt.float8e4
f32 = mybir.dt.float32
AF = mybir.ActivationFunctionType
ALU = mybir.AluOpType
DR = mybir.MatmulPerfMode.DoubleRow

P = 128
B, T, E, H, D = 4, 2048, 1024, 16, 64
F = 4 * E                    # 4096 MLP hidden
TQ = T // 2                  # 1024 own query tokens per core
NE = E // P                  # 8 e-chunks
NE2 = NE // 2                # 4 e-chunk DoubleRow pairs
NPAIR = H // 2               # 8 head pairs
NST = T // P                 # 16 context-token tiles
NST2 = NST // 2              # 8 context-tile DoubleRow pairs
NTS = TQ // P                # 8 own-token tiles
NF = F // P                  # 32 f-chunks
NF8 = 8                      # f-chunks computed in fp8 (first NF8)
NFB = NF - NF8               # f-chunks computed in bf16
NF82 = NF8 // 2
VW = D + 1                   # per-head V width incl. ones column
LN_EPS = 1e-5

S_H = 8.0                    # scale on LN outputs entering fp8
S_K = 16.0                   # scale on K stored fp8
S_VA = 16.0                  # scale on V entering fp8
S_CAT = 16.0                 # scale on attention output entering fp8
S_HID = 16.0                 # scale on relu(hid) entering fp8/bf16
SHIFT = math.log(128.0) - 8.0  # exp bias: P' = 128*exp(score-8)

_BUILD_CACHE = {}


class _Ctx:
    """Shared build state passed between phase emitters."""
    pass


def _emit_ln(g, xt, out_bf):
    nc = g.nc
    st = g.stat.tile([P, 2, nc.vector.BN_STATS_DIM], f32, name="bnst")
    xv = xt.rearrange("p (s g) -> p s g", s=2)
    nc.vector.bn_stats(out=st[:, 0, :], in_=xv[:, 0, :])
    nc.vector.bn_stats(out=st[:, 1, :], in_=xv[:, 1, :])
    mv = g.stat.tile([P, nc.vector.BN_AGGR_DIM], f32, name="bnmv")
    nc.vector.bn_aggr(out=mv, in_=st)
    rstd = g.stat.tile([P, 1], f32, name="bnrs")
    nc.scalar.activation(out=rstd, in_=mv[:, 1:2], func=AF.Sqrt, bias=g.eps_t)
    nc.vector.reciprocal(out=rstd, in_=rstd)
    nc.vector.tensor_scalar(
        out=out_bf, in0=xt, scalar1=mv[:, 0:1], scalar2=rstd,
        op0=ALU.subtract, op1=ALU.mult,
    )


def _emit_consts(g):
    nc, consts = g.nc, g.consts
    g.ident = consts.tile([P, P], bf16, name="ident")
    make_identity(nc, g.ident)
    g.eps_t = consts.tile([P, 1], f32, name="eps")
    nc.vector.memset(g.eps_t, LN_EPS)
    g.shift_t = consts.tile([P, 1], f32, name="shift")
    nc.vector.memset(g.shift_t, SHIFT)
    g.ub_sb = consts.tile([P, NF], f32, name="ubsb")
    nc.sync.dma_start(out=g.ub_sb, in_=g.ub_d[:, :])
    if g.has_qb:
        g.qb_sb = consts.tile([P, NPAIR], f32, name="qbsb")
        nc.sync.dma_start(out=g.qb_sb, in_=g.qb_d[:, :])
        g.kb_sb = consts.tile([P, NPAIR], f32, name="kbsb")
        nc.sync.dma_start(out=g.kb_sb, in_=g.kb_d[:, :])
        g.vb_bc = consts.tile([P, E], bf16, name="vbbc")
        nc.gpsimd.dma_start(
            out=g.vb_bc, in_=g.vbrow_d.ap()[0:1, :].partition_broadcast(P)[:, 0, :]
        )
    if g.has_pb:
        g.pb_bc = consts.tile([P, E], f32, name="pbbc")
        nc.gpsimd.dma_start(
            out=g.pb_bc, in_=g.pbrow_d.ap()[0:1, :].partition_broadcast(P)[:, 0, :]
        )
    if g.has_db:
        g.db_bc = consts.tile([P, E], f32, name="dbbc")
        nc.gpsimd.dma_start(
            out=g.db_bc, in_=g.dbrow_d.ap()[0:1, :].partition_broadcast(P)[:, 0, :]
        )


def _emit_ln1_transpose(g, xkp, tps):
    """Load x, LN1, PE-transpose h into e-major fp8 hT pairs (scaled by S_H)."""
    nc = g.nc
    for i in range(NST):
        xt = xkp.tile([P, E], f32, name="xk")
        nc.sync.dma_start(out=xt, in_=g.xkv_d[i * P:(i + 1) * P, :])
        ht = g.hp.tile([P, E], bf16, name="h")
        _emit_ln(g, xt, ht)
        for c in range(NE):
            tp = tps.tile([P, P], bf16, name="tp")
            nc.tensor.transpose(tp, ht[:, c * P:(c + 1) * P], g.ident)
            dst = g.hT2[c // 2][:, (c % 2) * T + i * P:(c % 2) * T + (i + 1) * P]
            nc.vector.tensor_scalar(
                out=dst, in0=tp, scalar1=S_H, scalar2=None, op0=ALU.mult)


def _emit_v(g, wvp, vps):
    """V in natural [s, d] fp8 layout (scale S_VA), ones column per head,
    context-tile pairs packed for DoubleRow attnV."""
    nc = g.nc
    wv_sb = []
    for c2 in range(NE2):
        w = wvp.tile([P, 2 * E], fp8, name=f"wv{c2}")
        nc.sync.dma_start(out=w, in_=g.wv_d[c2])
        wv_sb.append(w)
    for s2 in range(NST2):
        nc.gpsimd.dma_start(
            out=g.va2[s2],
            in_=g.vrow_d.ap()[0:1, :].partition_broadcast(P)[:, 0, :],
        )
        for sub in range(2):
            s = 2 * s2 + sub
            pv = [vps.tile([P, 512], f32, name=f"pv{j}") for j in range(2)]
            for c2 in range(NE2):
                hv = g.hT2[c2].rearrange("p (k t) -> p k t", k=2)
                wvv = wv_sb[c2].rearrange("p (k e) -> p k e", k=2)
                for j in range(2):
                    nc.tensor.matmul(
                        pv[j], hv[:, :, s * P:(s + 1) * P],
                        wvv[:, :, j * 512:(j + 1) * 512],
                        start=(c2 == 0), stop=(c2 == NE2 - 1), perf_mode=DR,
                    )
            vav = g.va2[s2].rearrange("p (k h v) -> p k h v", k=2, v=VW)
            for j in range(2):
                dst = vav[:, sub, j * 8:(j + 1) * 8, 0:D]
                src = pv[j].rearrange("p (h d) -> p h d", d=D)
                if g.has_qb:
                    vb_view = g.vb_bc.rearrange("p (h d) -> p h d", d=D)[
                        :, j * 8:(j + 1) * 8, :
                    ]
                    nc.vector.tensor_add(out=src, in0=src, in1=vb_view)
                nc.vector.tensor_scalar(
                    out=dst, in0=src, scalar1=g.c_va, scalar2=None, op0=ALU.mult)


def _emit_qkt_pair(g, p, qt, kt, wqkp, qkps):
    """Q^T (bf16) and K^T (fp8, scale S_K) for head pair p via DoubleRow."""
    nc = g.nc
    wq_sb = []
    wk_sb = []
    for c2 in range(NE2):
        wq = wqkp.tile([P, 2 * P], fp8, name="wq")
        nc.sync.dma_start(out=wq, in_=g.wq_d[c2, p])
        wq_sb.append(wq.rearrange("p (k m) -> p k m", k=2))
        wk = wqkp.tile([P, 2 * P], fp8, name="wk")
        nc.sync.dma_start(out=wk, in_=g.wk_d[c2, p])
        wk_sb.append(wk.rearrange("p (k m) -> p k m", k=2))
    for j in range(2):
        psq = qkps.tile([P, 512], f32, name="ps")
        for c2 in range(NE2):
            hv = g.hT2[c2].rearrange("p (k t) -> p k t", k=2)
            nc.tensor.matmul(
                psq, wq_sb[c2], hv[:, :, j * 512:(j + 1) * 512],
                start=(c2 == 0), stop=(c2 == NE2 - 1), perf_mode=DR,
            )
        dst = qt[:, j * 512:(j + 1) * 512]
        if g.has_qb:
            nc.vector.tensor_scalar(
                out=dst, in0=psq, scalar1=g.qb_sb[:, p:p + 1], op0=ALU.add)
        else:
            nc.vector.tensor_copy(out=dst, in_=psq)
    for j in range(4):
        psk = qkps.tile([P, 512], f32, name="ps")
        for c2 in range(NE2):
            hv = g.hT2[c2].rearrange("p (k t) -> p k t", k=2)
            nc.tensor.matmul(
                psk, wk_sb[c2], hv[:, :, j * 512:(j + 1) * 512],
                start=(c2 == 0), stop=(c2 == NE2 - 1), perf_mode=DR,
            )
        dst = kt[:, j * 512:(j + 1) * 512]
        if g.has_qb:
            nc.vector.tensor_scalar(
                out=dst, in0=psk, scalar1=g.kb_sb[:, p:p + 1], scalar2=g.c_k,
                op0=ALU.add, op1=ALU.mult)
        else:
            nc.vector.tensor_scalar(
                out=dst, in0=psk, scalar1=g.c_k, scalar2=None, op0=ALU.mult)


def _emit_attn_pair(g, p, th, qt, kt, ptp, smp, scps, atps):
    """Scores (transposed) -> fp8 P' = 128*exp(score-8) -> DoubleRow attnV.

    Covers query-half ``th`` (512 tokens) only."""
    nc = g.nc
    if True:
        tcols = slice(th * 512, (th + 1) * 512)
        at0 = atps.tile([VW, 512], f32, name="at0")
        at1 = atps.tile([VW, 512], f32, name="at1")
        for s2 in range(NST2):
            pt0 = ptp.tile([P, 2 * 512], fp8, name="pt0")
            pt1 = ptp.tile([P, 2 * 512], fp8, name="pt1")
            pt0v = pt0.rearrange("p (k q) -> p k q", k=2)
            pt1v = pt1.rearrange("p (k q) -> p k q", k=2)
            scA = scps.tile([P, 2 * 512], f32, name="scA")
            scB = scps.tile([P, 2 * 512], f32, name="scB")
            for sub in range(2):
                s = 2 * s2 + sub
                scols = slice(s * P, (s + 1) * P)
                sc = slice(sub * 512, (sub + 1) * 512)
                # two heads on PE row-groups 0-63 / 64-127: packed concurrent
                nc.tensor.matmul(scA[:, sc], kt[0:D, scols], qt[0:D, tcols],
                                 start=True, stop=True)
                nc.tensor.matmul(scB[:, sc], kt[D:2 * D, scols],
                                 qt[D:2 * D, tcols], start=True, stop=True)
            # one exp instruction per head covers both context subtiles
            nc.scalar.activation(out=pt0, in_=scA, func=AF.Exp,
                                 bias=g.shift_t, scale=g.c_s)
            nc.scalar.activation(out=pt1, in_=scB, func=AF.Exp,
                                 bias=g.shift_t, scale=g.c_s)
            vav = g.va2[s2].rearrange("p (k h v) -> p k h v", k=2, v=VW)
            nc.tensor.matmul(
                at0, vav[:, :, 2 * p, :], pt0v,
                start=(s2 == 0), stop=(s2 == NST2 - 1), perf_mode=DR,
            )
            nc.tensor.matmul(
                at1, vav[:, :, 2 * p + 1, :], pt1v,
                start=(s2 == 0), stop=(s2 == NST2 - 1), perf_mode=DR,
            )
        se0 = smp.tile([1, 512], f32, name="se0")
        se1 = smp.tile([1, 512], f32, name="se1")
        nc.vector.reciprocal(out=se0, in_=at0[D:D + 1, :])
        nc.vector.reciprocal(out=se1, in_=at1[D:D + 1, :])
        nc.vector.tensor_scalar(out=se0, in0=se0, scalar1=g.se_sc,
                                scalar2=None, op0=ALU.mult)
        nc.vector.tensor_scalar(out=se1, in0=se1, scalar1=g.se_sc,
                                scalar2=None, op0=ALU.mult)
        rb0 = smp.tile([D, 512], f32, name="rb0")
        rb1 = smp.tile([D, 512], f32, name="rb1")
        nc.gpsimd.partition_broadcast(rb0, se0)
        nc.gpsimd.partition_broadcast(rb1, se1)
        c2, half = p // 2, p % 2
        cc = slice(half * TQ + th * 512, half * TQ + (th + 1) * 512)
        nc.vector.tensor_mul(out=g.catT2[c2][0:D, cc], in0=at0[0:D, :], in1=rb0)
        nc.vector.tensor_mul(out=g.catT2[c2][D:2 * D, cc], in0=at1[0:D, :],
                             in1=rb1)


def _emit_proj_ln2(g, ts_range, xq2p, h2p, pps, t2ps):
    nc = g.nc
    pw_sb = g.pw_sb
    for ts in ts_range:
        trows = slice(ts * P, (ts + 1) * P)
        xres = xq2p.tile([P, E], f32, name="xres")
        nc.sync.dma_start(out=xres, in_=g.xkv_d[ts * P:(ts + 1) * P, :])
        x2 = g.x2_tiles[ts]
        for j in range(2):
            psy = pps.tile([P, 512], f32, name="py")
            for c2 in range(NE2):
                cv = g.catT2[c2].rearrange("p (k t) -> p k t", k=2)
                pwv = pw_sb[c2].rearrange("p (k e) -> p k e", k=2)
                nc.tensor.matmul(
                    psy, cv[:, :, trows], pwv[:, :, j * 512:(j + 1) * 512],
                    start=(c2 == 0), stop=(c2 == NE2 - 1), perf_mode=DR,
                )
            jc = slice(j * 512, (j + 1) * 512)
            nc.vector.tensor_scalar(out=x2[:, jc], in0=psy, scalar1=g.c_p,
                                    scalar2=None, op0=ALU.mult)
            if g.has_pb:
                nc.gpsimd.tensor_add(out=x2[:, jc], in0=x2[:, jc],
                                     in1=g.pb_bc[:, jc])
            nc.gpsimd.tensor_add(out=x2[:, jc], in0=x2[:, jc], in1=xres[:, jc])
        h2 = h2p.tile([P, E], bf16, name="h2")
        _emit_ln(g, x2, h2)
        for c in range(NE):
            tp = t2ps.tile([P, P], bf16, name="t2")
            nc.tensor.transpose(tp, h2[:, c * P:(c + 1) * P], g.ident)
            nc.vector.tensor_copy(out=g.h2Tb[c][:, trows], in_=tp)
            dst = g.h2T2[c // 2][:, (c % 2) * TQ + ts * P:(c % 2) * TQ + (ts + 1) * P]
            nc.vector.tensor_scalar(
                out=dst, in0=tp, scalar1=S_H, scalar2=None, op0=ALU.mult)


def _emit_up_unit(g, unit, uwtp, upps):
    """One up-projection unit for the first token half (512 tokens).

    unit = ('f8', f) for an fp8 chunk or ('fb', fb) for a bf16 chunk.
    Writes into the persistent hid tiles g.hid8_h[f // 2] / g.hidb_h[fb]."""
    nc = g.nc
    kind, f = unit
    pu = upps.tile([P, 512], f32, name="pu")
    if kind == 'f8':
        for c2 in range(NE2):
            h2v = g.h2T2[c2].rearrange("p (k t) -> p k t", k=2)
            nc.tensor.matmul(
                pu, g.uw8_sb[c2][:, :, f * P:(f + 1) * P], h2v[:, :, 0:512],
                start=(c2 == 0), stop=(c2 == NE2 - 1), perf_mode=DR,
            )
        hid8v = g.hid8_h[f // 2].rearrange("p (k t) -> p k t", k=2)
        nc.scalar.activation(out=hid8v[:, f % 2, :], in_=pu, func=AF.Relu,
                             bias=g.ub_sb[:, f:f + 1], scale=g.c_u)
    else:
        uwt = uwtp.tile([P, NE * P], bf16, name="uwt")
        nc.sync.dma_start(out=uwt, in_=g.uwb_d[f])
        for c in range(NE):
            nc.tensor.matmul(
                pu, uwt[:, c * P:(c + 1) * P], g.h2Tb[c][:, 0:512],
                start=(c == 0), stop=(c == NE - 1),
            )
        nc.scalar.activation(out=g.hidb_h[f], in_=pu, func=AF.Relu,
                             bias=g.ub_sb[:, NF8 + f:NF8 + f + 1], scale=S_HID)


def _emit_down_half(g, dwp8, dwpb, outp, dnps):
    """Down-projection for the first token half from the staged hid tiles."""
    nc = g.nc
    for grp in range(2):  # 256-token groups within the half
        gcols = slice(grp * 256, (grp + 1) * 256)
        dn = [dnps.tile([P, E], f32, name=f"dn{j}") for j in range(2)]
        for f2 in range(NF82):
            dwt = dwp8.tile([P, 2 * E], fp8, name="dwt8")
            nc.sync.dma_start(out=dwt, in_=g.dw8_d[f2])
            dwv = dwt.rearrange("p (k e) -> p k e", k=2)
            hid8v = g.hid8_h[f2].rearrange("p (k t) -> p k t", k=2)[:, :, gcols]
            for t2 in range(2):
                for j in range(2):
                    nc.tensor.matmul(
                        dn[t2][:, j * 512:(j + 1) * 512],
                        hid8v[:, :, t2 * P:(t2 + 1) * P],
                        dwv[:, :, j * 512:(j + 1) * 512],
                        start=(f2 == 0), stop=False, perf_mode=DR,
                    )
        for fb in range(NFB):
            dwt = dwpb.tile([P, E], bf16, name="dwtb")
            nc.sync.dma_start(out=dwt, in_=g.dwb_d[fb])
            hidv = g.hidb_h[fb][:, gcols]
            for t2 in range(2):
                for j in range(2):
                    nc.tensor.matmul(
                        dn[t2][:, j * 512:(j + 1) * 512],
                        hidv[:, t2 * P:(t2 + 1) * P],
                        dwt[:, j * 512:(j + 1) * 512],
                        start=False, stop=(fb == NFB - 1),
                    )
        for t2 in range(2):
            ti = grp * 2 + t2
            ot = outp.tile([P, E], f32, name="ot")
            nc.vector.tensor_scalar(out=ot, in0=dn[t2], scalar1=g.c_d,
                                    scalar2=None, op0=ALU.mult)
            if g.has_db:
                nc.gpsimd.tensor_add(out=ot, in0=ot, in1=g.db_bc)
            nc.gpsimd.tensor_add(out=ot, in0=ot, in1=g.x2_tiles[ti])
            nc.sync.dma_start(out=g.out_d[ti * P:(ti + 1) * P, :], in_=ot)


def _emit_mlp_fused(g, q, hidp8, hidpb, uwtp, dwp8, dwpb, outp, upps, dnps):
    """Fused up+down for one 256-token quarter (second token half)."""
    nc = g.nc
    TQQ = 256
    qcols = slice(q * TQQ, (q + 1) * TQQ)
    dn = [dnps.tile([P, E], f32, name=f"dn{j}") for j in range(2)]
    for f2 in range(NF82):
        hid8 = hidp8.tile([P, 2 * TQQ], fp8, name="hid8")
        hid8v = hid8.rearrange("p (k t) -> p k t", k=2)
        for sub in range(2):
            f = 2 * f2 + sub
            pu = upps.tile([P, TQQ], f32, name="puq")
            for c2 in range(NE2):
                h2v = g.h2T2[c2].rearrange("p (k t) -> p k t", k=2)
                nc.tensor.matmul(
                    pu, g.uw8_sb[c2][:, :, f * P:(f + 1) * P],
                    h2v[:, :, qcols],
                    start=(c2 == 0), stop=(c2 == NE2 - 1), perf_mode=DR,
                )
            nc.scalar.activation(out=hid8v[:, sub, :], in_=pu, func=AF.Relu,
                                 bias=g.ub_sb[:, f:f + 1], scale=g.c_u)
        dwt = dwp8.tile([P, 2 * E], fp8, name="dwt8")
        nc.sync.dma_start(out=dwt, in_=g.dw8_d[f2])
        dwv = dwt.rearrange("p (k e) -> p k e", k=2)
        for t2 in range(2):
            for j in range(2):
                nc.tensor.matmul(
                    dn[t2][:, j * 512:(j + 1) * 512],
                    hid8v[:, :, t2 * P:(t2 + 1) * P],
                    dwv[:, :, j * 512:(j + 1) * 512],
                    start=(f2 == 0), stop=False, perf_mode=DR,
                )
    for fb in range(NFB):
        uwt = uwtp.tile([P, NE * P], bf16, name="uwt")
        nc.sync.dma_start(out=uwt, in_=g.uwb_d[fb])
        pu = upps.tile([P, TQQ], f32, name="puq")
        for c in range(NE):
            nc.tensor.matmul(
                pu, uwt[:, c * P:(c + 1) * P], g.h2Tb[c][:, qcols],
                start=(c == 0), stop=(c == NE - 1),
            )
        hid = hidpb.tile([P, TQQ], bf16, name="hidb")
        nc.scalar.activation(out=hid, in_=pu, func=AF.Relu,
                             bias=g.ub_sb[:, NF8 + fb:NF8 + fb + 1],
                             scale=S_HID)
        dwt = dwpb.tile([P, E], bf16, name="dwtb")
        nc.sync.dma_start(out=dwt, in_=g.dwb_d[fb])
        for t2 in range(2):
            for j in range(2):
                nc.tensor.matmul(
                    dn[t2][:, j * 512:(j + 1) * 512],
                    hid[:, t2 * P:(t2 + 1) * P],
                    dwt[:, j * 512:(j + 1) * 512],
                    start=False, stop=(fb == NFB - 1),
                )
    for t2 in range(2):
        ti = q * 2 + t2
        ot = outp.tile([P, E], f32, name="ot")
        nc.vector.tensor_scalar(out=ot, in0=dn[t2], scalar1=g.c_d,
                                scalar2=None, op0=ALU.mult)
        if g.has_db:
            nc.gpsimd.tensor_add(out=ot, in0=ot, in1=g.db_bc)
        nc.gpsimd.tensor_add(out=ot, in0=ot, in1=g.x2_tiles[ti])
        nc.sync.dma_start(out=g.out_d[ti * P:(ti + 1) * P, :], in_=ot)


def _build(flags, sexp):
    has_qb, has_pb, has_db = flags
    e_wq, e_wk, e_wv, e_pw, e_uw, e_dw = sexp
    s_wq, s_wk = 2.0 ** e_wq, 2.0 ** e_wk
    s_wv, s_pw = 2.0 ** e_wv, 2.0 ** e_pw
    s_uw, s_dw = 2.0 ** e_uw, 2.0 ** e_dw
    nc = bacc.Bacc("TRN2", target_bir_lowering=False, debug=False, num_devices=8)

    g = _Ctx()
    g.nc = nc
    g.has_qb, g.has_pb, g.has_db = flags
    g.c_s = 1.0 / (S_K * S_H * s_wq)          # exp input dequant
    g.c_k = S_K / (S_H * s_wk)                # K psum -> fp8 kt
    g.c_va = S_VA / (S_H * s_wv)              # V psum -> fp8 va
    g.se_sc = S_CAT / S_VA                    # folded into 1/denom
    g.c_p = 1.0 / (S_CAT * s_pw)              # proj psum dequant
    g.c_u = S_HID / (S_H * s_uw)              # up psum -> relu input scale
    g.c_d = 1.0 / (S_HID * s_dw)              # down psum dequant
    g.xkv_d = nc.dram_tensor("xkv", [T, E], f32, kind="ExternalInput")
    g.wq_d = nc.dram_tensor("wq", [NE2, NPAIR, P, 2 * P], fp8,
                            kind="ExternalInput")
    g.wk_d = nc.dram_tensor("wk", [NE2, NPAIR, P, 2 * P], fp8,
                            kind="ExternalInput")
    g.wv_d = nc.dram_tensor("wv", [NE2, P, 2 * E], fp8, kind="ExternalInput")
    g.vrow_d = nc.dram_tensor("vrow", [1, 2 * H * VW], fp8,
                              kind="ExternalInput")
    g.pw_d = nc.dram_tensor("pw", [NE2, P, 2 * E], fp8, kind="ExternalInput")
    g.uw8_d = nc.dram_tensor("uw8", [NE2, P, 2 * NF8 * P], fp8,
                             kind="ExternalInput")
    g.uwb_d = nc.dram_tensor("uwb", [NFB, P, NE * P], bf16,
                             kind="ExternalInput")
    g.ub_d = nc.dram_tensor("ub", [P, NF], f32, kind="ExternalInput")
    g.dw8_d = nc.dram_tensor("dw8", [NF82, P, 2 * E], fp8,
                             kind="ExternalInput")
    g.dwb_d = nc.dram_tensor("dwb", [NFB, P, E], bf16, kind="ExternalInput")
    if has_qb:
        g.qb_d = nc.dram_tensor("qb", [P, NPAIR], f32, kind="ExternalInput")
        g.kb_d = nc.dram_tensor("kb", [P, NPAIR], f32, kind="ExternalInput")
        g.vbrow_d = nc.dram_tensor("vbrow", [1, E], bf16, kind="ExternalInput")
    if has_pb:
        g.pbrow_d = nc.dram_tensor("pbrow", [1, E], f32, kind="ExternalInput")
    if has_db:
        g.dbrow_d = nc.dram_tensor("dbrow", [1, E], f32, kind="ExternalInput")
    g.out_d = nc.dram_tensor("out", [TQ, E], f32, kind="ExternalOutput")

    with tile.TileContext(nc) as tc:
        with (
            tc.tile_pool(name="consts", bufs=1) as consts,
            tc.tile_pool(name="stat", bufs=4) as stat,
            tc.tile_pool(name="catp", bufs=1) as catp,
            tc.tile_pool(name="x2p", bufs=1) as x2p,
            tc.tile_pool(name="h2Tp", bufs=1) as h2Tp,
        ):
            g.consts, g.stat = consts, stat
            _emit_consts(g)
            _emit_all(g, tc, catp, x2p, h2Tp)

    nc.finalize()
    return nc


def _emit_all(g, tc, catp, x2p, h2Tp):
    g.catT2 = [catp.tile([P, 2 * TQ], fp8, name=f"catT{c2}")
               for c2 in range(NE2)]
    g.x2_tiles = [x2p.tile([P, E], f32, name=f"x2_{i}") for i in range(NTS)]
    g.h2T2 = [h2Tp.tile([P, 2 * TQ], fp8, name=f"h2T8{c2}")
              for c2 in range(NE2)]
    g.h2Tb = [h2Tp.tile([P, TQ], bf16, name=f"h2Tb{c}") for c in range(NE)]

    g.pwp = tc.alloc_tile_pool(name="pwp", bufs=1)
    g.uwp = tc.alloc_tile_pool(name="uwp", bufs=1)
    g.hidhp = tc.alloc_tile_pool(name="hidh", bufs=1)
    g.hid8_h = [g.hidhp.tile([P, 2 * 512], fp8, name=f"h8h{f2}")
                for f2 in range(NF82)]
    g.hidb_h = [g.hidhp.tile([P, 512], bf16, name=f"hbh{fb}")
                for fb in range(NFB)]

    with (
        tc.tile_pool(name="vaug", bufs=1) as vap,
        tc.tile_pool(name="qtp", bufs=1) as qtp,
        tc.tile_pool(name="ktp", bufs=1) as ktp,
    ):
        g.va2 = [vap.tile([P, 2 * H * VW], fp8, name=f"va{s2}")
                 for s2 in range(NST2)]
        qts = [qtp.tile([P, TQ], bf16, name=f"qt{p}") for p in range(NPAIR)]
        kts = [ktp.tile([P, T], fp8, name=f"kt{p}") for p in range(NPAIR)]

        with (
            tc.tile_pool(name="hp", bufs=4) as hp,
            tc.tile_pool(name="hTp", bufs=1) as hTp,
        ):
            g.hp = hp
            g.hT2 = [hTp.tile([P, 2 * T], fp8, name=f"hT{c2}")
                     for c2 in range(NE2)]
            with (
                tc.tile_pool(name="xk", bufs=3) as xkp,
                tc.tile_pool(name="tps", bufs=2, space="PSUM") as tps,
            ):
                _emit_ln1_transpose(g, xkp, tps)
            with (
                tc.tile_pool(name="wvp", bufs=1) as wvp,
                tc.tile_pool(name="vps", bufs=4, space="PSUM") as vps,
            ):
                _emit_v(g, wvp, vps)
            with (
                tc.tile_pool(name="wqk", bufs=10) as wqkp,
                tc.tile_pool(name="qkps", bufs=2, space="PSUM") as qkps,
            ):
                for p in range(NPAIR):
                    _emit_qkt_pair(g, p, qts[p], kts[p], wqkp, qkps)
        # hT2 freed here (16KB back) before the attention phase

        with (
            tc.tile_pool(name="ptp", bufs=4) as ptp,
            tc.tile_pool(name="smp", bufs=2) as smp,
            tc.tile_pool(name="atps", bufs=1, space="PSUM") as atps,
            tc.tile_pool(name="scps", bufs=1, space="PSUM") as scps,
        ):
            # query-half 0 attention (no MLP work ready to overlap yet)
            for p in range(NPAIR):
                _emit_attn_pair(g, p, 0, qts[p], kts[p], ptp, smp, scps, atps)
                if p == 0:
                    # prefetch proj/up weights on the idle SWDGE queue
                    g.pw_sb = []
                    for c2 in range(NE2):
                        w = g.pwp.tile([P, 2 * E], fp8, name=f"pw{c2}")
                        g.nc.gpsimd.dma_start(out=w, in_=g.pw_d[c2])
                        g.pw_sb.append(w)
                if p == 1:
                    g.uw8_sb = []
                    for c2 in range(NE2):
                        w = g.uwp.tile([P, 2 * NF8 * P], fp8, name=f"uw8{c2}")
                        g.nc.gpsimd.dma_start(out=w, in_=g.uw8_d[c2])
                        g.uw8_sb.append(w.rearrange("p (k m) -> p k m", k=2))
            # query-half 1 attention overlapped with proj/LN2 + up of half 0
            with (
                tc.tile_pool(name="xq2", bufs=3) as xq2p,
                tc.tile_pool(name="h2p", bufs=3) as h2p,
            ):
                with tc.tile_pool(name="pps", bufs=1, space="PSUM") as pps:
                    _emit_attn_pair(g, 0, 1, qts[0], kts[0], ptp, smp, scps,
                                    atps)
                    _emit_attn_pair(g, 1, 1, qts[1], kts[1], ptp, smp, scps,
                                    atps)
                    _emit_proj_ln2(g, range(0, 4), xq2p, h2p, pps, pps)
                with (
                    tc.tile_pool(name="uwtp", bufs=3) as uwtp,
                    tc.tile_pool(name="upps", bufs=2, space="PSUM") as upps,
                ):
                    units = ([('f8', f) for f in range(NF8)]
                             + [('fb', fb) for fb in range(NFB)])
                    ui = 0
                    for i, p in enumerate(range(2, NPAIR)):
                        _emit_attn_pair(g, p, 1, qts[p], kts[p], ptp, smp,
                                        scps, atps)
                        take = (len(units) * (i + 1)) // 6 - ui
                        for unit in units[ui:ui + take]:
                            _emit_up_unit(g, unit, uwtp, upps)
                        ui += take

    # tail: proj/LN2 of half 1, down of half 0, fused MLP of half 1
    with (
        tc.tile_pool(name="xq2", bufs=3) as xq2p,
        tc.tile_pool(name="h2p", bufs=3) as h2p,
        tc.tile_pool(name="hidp8", bufs=3) as hidp8,
        tc.tile_pool(name="hidpb", bufs=6) as hidpb,
        tc.tile_pool(name="uwtp", bufs=3) as uwtp,
        tc.tile_pool(name="dwp8", bufs=2) as dwp8,
        tc.tile_pool(name="dwpb", bufs=4) as dwpb,
        tc.tile_pool(name="outp", bufs=3) as outp,
        tc.tile_pool(name="pps", bufs=1, space="PSUM") as pps,
        tc.tile_pool(name="upps", bufs=2, space="PSUM") as upps,
        tc.tile_pool(name="dnps", bufs=1, space="PSUM") as dnps,
    ):
        _emit_proj_ln2(g, range(4, 8), xq2p, h2p, pps, pps)
        _emit_down_half(g, dwp8, dwpb, outp, dnps)
        _emit_mlp_fused(g, 2, hidp8, hidpb, uwtp, dwp8, dwpb, outp, upps,
                        dnps)
        _emit_mlp_fused(g, 3, hidp8, hidpb, uwtp, dwp8, dwpb, outp, upps,
                        dnps)
    g.hidhp.release()
    g.uwp.release()
    g.pwp.release()


def _get_nc(flags, sexp):
    key = (flags, sexp)
    if key not in _BUILD_CACHE:
        _BUILD_CACHE[key] = _build(flags, sexp)
    return _BUILD_CACHE[key]


def _po2_exp(w):
    """Power-of-2 exponent e such that |w|*2^e lands in (112, 224]."""
    amax = float(np.abs(w).max())
    if amax == 0.0:
        return 0
    return int(np.floor(np.log2(224.0 / amax)))


def _prep(x, Wq, Wk, Wv, proj_w, proj_b, ln1_g, ln1_b, ln2_g, ln2_b,
          up_w, up_b, down_w, down_b):
    """Host-side shard + weight fold/quantize/layout.

    Returns (flags, sexp, in_maps)."""
    bfl = ml_dtypes.bfloat16
    f8l = ml_dtypes.float8_e4m3
    x = np.ascontiguousarray(np.asarray(x, dtype=np.float32))
    Wq = np.asarray(Wq, np.float32)
    Wk = np.asarray(Wk, np.float32)
    Wv = np.asarray(Wv, np.float32)
    g1 = np.asarray(ln1_g, np.float32)
    b1 = np.asarray(ln1_b, np.float32)
    g2 = np.asarray(ln2_g, np.float32)
    b2 = np.asarray(ln2_b, np.float32)
    proj_w = np.asarray(proj_w, np.float32)
    up_w = np.asarray(up_w, np.float32)
    down_w = np.asarray(down_w, np.float32)

    # [H, E, D] -> [E, H*D]; fold attention scale into Q, LN1 gain into all
    wq_all = (Wq * (D ** -0.5)).transpose(1, 0, 2).reshape(E, E)
    wk_all = Wk.transpose(1, 0, 2).reshape(E, E)
    wv_all = Wv.transpose(1, 0, 2).reshape(E, E)
    qb_vec = b1 @ wq_all
    kb_vec = b1 @ wk_all
    vb_vec = b1 @ wv_all
    wq_f = g1[:, None] * wq_all
    wk_f = g1[:, None] * wk_all
    wv_f = g1[:, None] * wv_all
    uw_f = g2[:, None] * up_w
    ub_f = np.asarray(up_b, np.float32) + b2 @ up_w

    e_wq, e_wk, e_wv = _po2_exp(wq_f), _po2_exp(wk_f), _po2_exp(wv_f)
    e_pw, e_uw, e_dw = _po2_exp(proj_w), _po2_exp(uw_f), _po2_exp(down_w)
    sexp = (e_wq, e_wk, e_wv, e_pw, e_uw, e_dw)

    def _qkpair_chunks(w, e):  # [E, E] -> [NE2, NPAIR, P, 2*P] fp8
        ws = (w * 2.0 ** e).reshape(NE2, 2, P, NPAIR, P)
        return np.ascontiguousarray(
            ws.transpose(0, 3, 2, 1, 4).reshape(NE2, NPAIR, P, 2 * P)
            .astype(f8l))

    def _kpair(w, e, ncols):  # [E, ncols] -> [NE2, P, 2*ncols] fp8
        ws = (w * 2.0 ** e).reshape(NE2, 2, P, ncols)
        return np.ascontiguousarray(
            ws.transpose(0, 2, 1, 3).reshape(NE2, P, 2 * ncols).astype(f8l))

    vrow = np.zeros((1, 2 * H * VW), np.float32)
    vrow.reshape(2, H, VW)[:, :, D] = 1.0

    # down: first NF8 chunks fp8 (k-pairs), rest bf16 pre-scaled by s_dw
    dw_s = down_w * 2.0 ** e_dw
    dw8 = np.ascontiguousarray(
        dw_s[:NF8 * P].reshape(NF82, 2, P, E).transpose(0, 2, 1, 3)
        .reshape(NF82, P, 2 * E).astype(f8l))
    dwb = np.ascontiguousarray(
        dw_s[NF8 * P:].reshape(NFB, P, E).astype(bfl))
    uw8 = _kpair(uw_f[:, :NF8 * P], e_uw, NF8 * P)
    # bf16 up chunks: [NFB, P(e within chunk), NE*P] so one DMA per f-chunk
    uwb = np.ascontiguousarray(
        uw_f[:, NF8 * P:].reshape(NE, P, NFB, P).transpose(2, 1, 0, 3)
        .reshape(NFB, P, NE * P).astype(bfl))

    has_qb = bool(np.any(b1 != 0))
    has_pb = bool(np.any(np.asarray(proj_b) != 0))
    has_db = bool(np.any(np.asarray(down_b) != 0))
    flags = (has_qb, has_pb, has_db)

    shared = {
        "wq": _qkpair_chunks(wq_f, e_wq),
        "wk": _qkpair_chunks(wk_f, e_wk),
        "wv": _kpair(wv_f, e_wv, E),
        "vrow": vrow.astype(f8l),
        "pw": _kpair(proj_w, e_pw, E),
        "uw8": uw8,
        "uwb": uwb,
        "ub": np.ascontiguousarray(
            (S_HID * ub_f).reshape(NF, P).T.astype(np.float32)),
        "dw8": dw8,
        "dwb": dwb,
    }
    if has_qb:
        shared["qb"] = np.ascontiguousarray(
            (qb_vec * (S_H * 2.0 ** e_wq)).reshape(NPAIR, P).T
            .astype(np.float32))
        shared["kb"] = np.ascontiguousarray(
            (kb_vec * (S_H * 2.0 ** e_wk)).reshape(NPAIR, P).T
            .astype(np.float32))
        shared["vbrow"] = (vb_vec * (S_H * 2.0 ** e_wv)).reshape(1, E)\
            .astype(bfl)
    if has_pb:
        shared["pbrow"] = np.asarray(proj_b, np.float32).reshape(1, E)
    if has_db:
        shared["dbrow"] = np.asarray(down_b, np.float32).reshape(1, E)

    in_maps = []
    for c in range(8):
        b, half = c // 2, c % 2
        xb = x[b]
        if half == 1:
            xb = np.concatenate([xb[TQ:], xb[:TQ]], axis=0)
        in_maps.append({"xkv": np.ascontiguousarray(xb), **shared})
    return flags, sexp, in_maps


def kernel(**inputs) -> np.ndarray:
    flags, sexp, in_maps = _prep(**inputs)
    nc = _get_nc(flags, sexp)
    res = run_bass_kernel_spmd(nc, in_maps, core_ids=list(range(8)))
    out = np.empty((B, T, E), np.float32)
    for c in range(8):
        b, half = c // 2, c % 2
        out[b, half * TQ:(half + 1) * TQ, :] = res.results[c]["out"]
    return out


# revision 29
# speedup vs baseline: 1.3333x; 1.3333x over previous
"""Trainium2 Bass kernel for a dense transformer block (nn_Block_120259084502).

Contract: kernel(**inputs) takes the FULL unsharded inputs (numpy, fp32) and
returns the FULL output [4, 2048, 1024] fp32. Internally shards across 8
NeuronCores: core c handles batch c//2, query-token half c%2. Each core
receives its batch's full 2048 tokens (rolled so its own 1024 query tokens
come first) and computes K/V for all of them locally, so no collectives are
needed (attention context = full batch; softmax is order-invariant so the
roll is harmless).

Precision strategy (validated against the reference on the host):
- The attention path runs in fp8 e4m3 with DoubleRow matmuls (2 rows/cycle):
  LN1 output h, Wq/Wk/Wv, V, the softmax probabilities P, cat and proj_w.
  P is written as exp(score*c - 8 + ln 128) directly in e4m3 by the scalar
  engine; the constant shift cancels exactly in the softmax normalization.
- The MLP runs the first NF8 f-chunks in fp8-DoubleRow and the rest in bf16;
  bf16 dw/hid are pre-scaled by the same power-of-two factors so both
  accumulate into one PSUM group with a single final dequant.
All scales are powers of two (exact), weight scales picked from host absmax.
LayerNorm statistics and both residual adds stay fp32.
"""

import math

import numpy as np
import ml_dtypes

import concourse.bacc as bacc
import concourse.tile as tile
from concourse import mybir
from concourse.bass_utils import run_bass_kernel_spmd
from concourse.masks import make_identity

bf16 = mybir.dt.bfloat16
fp8 = mybir.dt.float8e4
f32 = mybir.dt.float32
AF = mybir.ActivationFunctionType
ALU = mybir.AluOpType
DR = mybir.MatmulPerfMode.DoubleRow

P = 128
B, T, E, H, D = 4, 2048, 1024, 16, 64
F = 4 * E                    # 4096 MLP hidden
TQ = T // 2                  # 1024 own query tokens per core
NE = E // P                  # 8 e-chunks
NE2 = NE // 2                # 4 e-chunk DoubleRow pairs
NPAIR = H // 2               # 8 head pairs
NST = T // P                 # 16 context-token tiles
NST2 = NST // 2              # 8 context-tile DoubleRow pairs
NTS = TQ // P                # 8 own-token tiles
NF = F // P                  # 32 f-chunks
NF8 = 8                      # f-chunks computed in fp8 (first NF8)
NFB = NF - NF8               # f-chunks computed in bf16
NF82 = NF8 // 2
VW = D + 1                   # per-head V width incl. ones column
LN_EPS = 1e-5

S_H = 8.0                    # scale on LN outputs entering fp8
S_K = 16.0                   # scale on K stored fp8
S_VA = 16.0                  # scale on V entering fp8
S_CAT = 16.0                 # scale on attention output entering fp8
S_HID = 16.0                 # scale on relu(hid) entering fp8/bf16
SHIFT = math.log(128.0) - 8.0  # exp bias: P' = 128*exp(score-8)

_BUILD_CACHE = {}


class _Ctx:
    """Shared build state passed between phase emitters."""
    pass


def _emit_ln(g, xt, out_bf):
    nc = g.nc
    st = g.stat.tile([P, 2, nc.vector.BN_STATS_DIM], f32, name="bnst")
    xv = xt.rearrange("p (s g) -> p s g", s=2)
    nc.vector.bn_stats(out=st[:, 0, :], in_=xv[:, 0, :])
    nc.vector.bn_stats(out=st[:, 1, :], in_=xv[:, 1, :])
    mv = g.stat.tile([P, nc.vector.BN_AGGR_DIM], f32, name="bnmv")
    nc.vector.bn_aggr(out=mv, in_=st)
    rstd = g.stat.tile([P, 1], f32, name="bnrs")
    nc.scalar.activation(out=rstd, in_=mv[:, 1:2], func=AF.Sqrt, bias=g.eps_t)
    nc.vector.reciprocal(out=rstd, in_=rstd)
    nc.vector.tensor_scalar(
        out=out_bf, in0=xt, scalar1=mv[:, 0:1], scalar2=rstd,
        op0=ALU.subtract, op1=ALU.mult,
    )


def _emit_consts(g):
    nc, consts = g.nc, g.consts
    g.ident = consts.tile([P, P], bf16, name="ident")
    make_identity(nc, g.ident)
    g.eps_t = consts.tile([P, 1], f32, name="eps")
    nc.vector.memset(g.eps_t, LN_EPS)
    g.shift_t = consts.tile([P, 1], f32, name="shift")
    nc.vector.memset(g.shift_t, SHIFT)
    g.ub_sb = consts.tile([P, NF], f32, name="ubsb")
    nc.sync.dma_start(out=g.ub_sb, in_=g.ub_d[:, :])
    if g.has_qb:
        g.qb_sb = consts.tile([P, NPAIR], f32, name="qbsb")
        nc.sync.dma_start(out=g.qb_sb, in_=g.qb_d[:, :])
        g.kb_sb = consts.tile([P, NPAIR], f32, name="kbsb")
        nc.sync.dma_start(out=g.kb_sb, in_=g.kb_d[:, :])
        g.vb_bc = consts.tile([P, E], bf16, name="vbbc")
        nc.gpsimd.dma_start(
            out=g.vb_bc, in_=g.vbrow_d.ap()[0:1, :].partition_broadcast(P)[:, 0, :]
        )
    if g.has_pb:
        g.pb_bc = consts.tile([P, E], f32, name="pbbc")
        nc.gpsimd.dma_start(
            out=g.pb_bc, in_=g.pbrow_d.ap()[0:1, :].partition_broadcast(P)[:, 0, :]
        )
    if g.has_db:
        g.db_bc = consts.tile([P, E], f32, name="dbbc")
        nc.gpsimd.dma_start(
            out=g.db_bc, in_=g.dbrow_d.ap()[0:1, :].partition_broadcast(P)[:, 0, :]
        )


def _emit_ln1_transpose(g, xkp, tps):
    """Load x, LN1, PE-transpose h into e-major fp8 hT pairs (scaled by S_H)."""
    nc = g.nc
    for i in range(NST):
        xt = xkp.tile([P, E], f32, name="xk")
        nc.sync.dma_start(out=xt, in_=g.xkv_d[i * P:(i + 1) * P, :])
        ht = g.hp.tile([P, E], bf16, name="h")
        _emit_ln(g, xt, ht)
        for c in range(NE):
            tp = tps.tile([P, P], bf16, name="tp")
            nc.tensor.transpose(tp, ht[:, c * P:(c + 1) * P], g.ident)
            dst = g.hT2[c // 2][:, (c % 2) * T + i * P:(c % 2) * T + (i + 1) * P]
            nc.vector.tensor_scalar(
                out=dst, in0=tp, scalar1=S_H, scalar2=None, op0=ALU.mult)


def _emit_v(g, wvp, vps):
    """V in natural [s, d] fp8 layout (scale S_VA), ones column per head,
    context-tile pairs packed for DoubleRow attnV."""
    nc = g.nc
    wv_sb = []
    for c2 in range(NE2):
        w = wvp.tile([P, 2 * E], fp8, name=f"wv{c2}")
        nc.sync.dma_start(out=w, in_=g.wv_d[c2])
        wv_sb.append(w)
    for s2 in range(NST2):
        nc.gpsimd.dma_start(
            out=g.va2[s2],
            in_=g.vrow_d.ap()[0:1, :].partition_broadcast(P)[:, 0, :],
        )
        for sub in range(2):
            s = 2 * s2 + sub
            pv = [vps.tile([P, 512], f32, name=f"pv{j}") for j in range(2)]
            for c2 in range(NE2):
                hv = g.hT2[c2].rearrange("p (k t) -> p k t", k=2)
                wvv = wv_sb[c2].rearrange("p (k e) -> p k e", k=2)
                for j in range(2):
                    nc.tensor.matmul(
                        pv[j], hv[:, :, s * P:(s + 1) * P],
                        wvv[:, :, j * 512:(j + 1) * 512],
                        start=(c2 == 0), stop=(c2 == NE2 - 1), perf_mode=DR,
                    )
            vav = g.va2[s2].rearrange("p (k h v) -> p k h v", k=2, v=VW)
            for j in range(2):
                dst = vav[:, sub, j * 8:(j + 1) * 8, 0:D]
                src = pv[j].rearrange("p (h d) -> p h d", d=D)
                if g.has_qb:
                    vb_view = g.vb_bc.rearrange("p (h d) -> p h d", d=D)[
                        :, j * 8:(j + 1) * 8, :
                    ]
                    nc.vector.tensor_add(out=src, in0=src, in1=vb_view)
                nc.vector.tensor_scalar(
                    out=dst, in0=src, scalar1=g.c_va, scalar2=None, op0=ALU.mult)


def _emit_qkt_pair(g, p, qt, kt, wqkp, qkps):
    """Q^T (bf16) and K^T (fp8, scale S_K) for head pair p via DoubleRow."""
    nc = g.nc
    wq_sb = []
    wk_sb = []
    for c2 in range(NE2):
        wq = wqkp.tile([P, 2 * P], fp8, name="wq")
        nc.sync.dma_start(out=wq, in_=g.wq_d[c2, p])
        wq_sb.append(wq.rearrange("p (k m) -> p k m", k=2))
        wk = wqkp.tile([P, 2 * P], fp8, name="wk")
        nc.sync.dma_start(out=wk, in_=g.wk_d[c2, p])
        wk_sb.append(wk.rearrange("p (k m) -> p k m", k=2))
    for j in range(2):
        psq = qkps.tile([P, 512], f32, name="ps")
        for c2 in range(NE2):
            hv = g.hT2[c2].rearrange("p (k t) -> p k t", k=2)
            nc.tensor.matmul(
                psq, wq_sb[c2], hv[:, :, j * 512:(j + 1) * 512],
                start=(c2 == 0), stop=(c2 == NE2 - 1), perf_mode=DR,
            )
        dst = qt[:, j * 512:(j + 1) * 512]
        if g.has_qb:
            nc.vector.tensor_scalar(
                out=dst, in0=psq, scalar1=g.qb_sb[:, p:p + 1], op0=ALU.add)
        else:
            nc.vector.tensor_copy(out=dst, in_=psq)
    for j in range(4):
        psk = qkps.tile([P, 512], f32, name="ps")
        for c2 in range(NE2):
            hv = g.hT2[c2].rearrange("p (k t) -> p k t", k=2)
            nc.tensor.matmul(
                psk, wk_sb[c2], hv[:, :, j * 512:(j + 1) * 512],
                start=(c2 == 0), stop=(c2 == NE2 - 1), perf_mode=DR,
            )
        dst = kt[:, j * 512:(j + 1) * 512]
        if g.has_qb:
            nc.vector.tensor_scalar(
                out=dst, in0=psk, scalar1=g.kb_sb[:, p:p + 1], scalar2=g.c_k,
                op0=ALU.add, op1=ALU.mult)
        else:
            nc.vector.tensor_scalar(
                out=dst, in0=psk, scalar1=g.c_k, scalar2=None, op0=ALU.mult)


def _emit_attn_pair(g, p, th, qt, kt, ptp, smp, scps, atps, fillers=None):
    """Scores (transposed) -> fp8 P' = 128*exp(score-8) -> DoubleRow attnV.

    Covers query-half ``th`` (512 tokens) only. ``fillers`` is a list of
    zero-arg emit callables; one is popped and emitted after each context
    step so independent PE work fills the exp-wait bubbles."""
    nc = g.nc
    if True:
        tcols = slice(th * 512, (th + 1) * 512)
        at0 = atps.tile([VW, 512], f32, name="at0")
        at1 = atps.tile([VW, 512], f32, name="at1")
        for s2 in range(NST2):
            pt0 = ptp.tile([P, 2 * 512], fp8, name="pt0")
            pt1 = ptp.tile([P, 2 * 512], fp8, name="pt1")
            pt0v = pt0.rearrange("p (k q) -> p k q", k=2)
            pt1v = pt1.rearrange("p (k q) -> p k q", k=2)
            scA = scps.tile([P, 2 * 512], f32, name="scA")
            scB = scps.tile([P, 2 * 512], f32, name="scB")
            for sub in range(2):
                s = 2 * s2 + sub
                scols = slice(s * P, (s + 1) * P)
                sc = slice(sub * 512, (sub + 1) * 512)
                # two heads on PE row-groups 0-63 / 64-127: packed concurrent
                nc.tensor.matmul(scA[:, sc], kt[0:D, scols], qt[0:D, tcols],
                                 start=True, stop=True)
                nc.tensor.matmul(scB[:, sc], kt[D:2 * D, scols],
                                 qt[D:2 * D, tcols], start=True, stop=True)
            # one exp instruction per head covers both context subtiles
            nc.scalar.activation(out=pt0, in_=scA, func=AF.Exp,
                                 bias=g.shift_t, scale=g.c_s)
            nc.scalar.activation(out=pt1, in_=scB, func=AF.Exp,
                                 bias=g.shift_t, scale=g.c_s)
            vav = g.va2[s2].rearrange("p (k h v) -> p k h v", k=2, v=VW)
            nc.tensor.matmul(
                at0, vav[:, :, 2 * p, :], pt0v,
                start=(s2 == 0), stop=(s2 == NST2 - 1), perf_mode=DR,
            )
            nc.tensor.matmul(
                at1, vav[:, :, 2 * p + 1, :], pt1v,
                start=(s2 == 0), stop=(s2 == NST2 - 1), perf_mode=DR,
            )
            if fillers:
                fillers.pop(0)()
        se0 = smp.tile([1, 512], f32, name="se0")
        se1 = smp.tile([1, 512], f32, name="se1")
        nc.vector.reciprocal(out=se0, in_=at0[D:D + 1, :])
        nc.vector.reciprocal(out=se1, in_=at1[D:D + 1, :])
        nc.vector.tensor_scalar(out=se0, in0=se0, scalar1=g.se_sc,
                                scalar2=None, op0=ALU.mult)
        nc.vector.tensor_scalar(out=se1, in0=se1, scalar1=g.se_sc,
                                scalar2=None, op0=ALU.mult)
        rb0 = smp.tile([D, 512], f32, name="rb0")
        rb1 = smp.tile([D, 512], f32, name="rb1")
        nc.gpsimd.partition_broadcast(rb0, se0)
        nc.gpsimd.partition_broadcast(rb1, se1)
        c2, half = p // 2, p % 2
        cc = slice(half * TQ + th * 512, half * TQ + (th + 1) * 512)
        nc.vector.tensor_mul(out=g.catT2[c2][0:D, cc], in0=at0[0:D, :], in1=rb0)
        nc.vector.tensor_mul(out=g.catT2[c2][D:2 * D, cc], in0=at1[0:D, :],
                             in1=rb1)


def _emit_proj_ln2(g, ts_range, xq2p, h2p, pps, t2ps):
    for ts in ts_range:
        _emit_proj_ts(g, ts, xq2p, h2p, pps, t2ps)


def _emit_proj_ts(g, ts, xq2p, h2p, pps, t2ps):
    nc = g.nc
    pw_sb = g.pw_sb
    if True:
        trows = slice(ts * P, (ts + 1) * P)
        xres = xq2p.tile([P, E], f32, name="xres")
        nc.sync.dma_start(out=xres, in_=g.xkv_d[ts * P:(ts + 1) * P, :])
        x2 = g.x2_tiles[ts]
        for j in range(2):
            psy = pps.tile([P, 512], f32, name="py")
            for c2 in range(NE2):
                cv = g.catT2[c2].rearrange("p (k t) -> p k t", k=2)
                pwv = pw_sb[c2].rearrange("p (k e) -> p k e", k=2)
                nc.tensor.matmul(
                    psy, cv[:, :, trows], pwv[:, :, j * 512:(j + 1) * 512],
                    start=(c2 == 0), stop=(c2 == NE2 - 1), perf_mode=DR,
                )
            jc = slice(j * 512, (j + 1) * 512)
            nc.vector.tensor_scalar(out=x2[:, jc], in0=psy, scalar1=g.c_p,
                                    scalar2=None, op0=ALU.mult)
            if g.has_pb:
                nc.gpsimd.tensor_add(out=x2[:, jc], in0=x2[:, jc],
                                     in1=g.pb_bc[:, jc])
            nc.gpsimd.tensor_add(out=x2[:, jc], in0=x2[:, jc], in1=xres[:, jc])
        h2 = h2p.tile([P, E], bf16, name="h2")
        _emit_ln(g, x2, h2)
        for c in range(NE):
            tp = t2ps.tile([P, P], bf16, name="t2")
            nc.tensor.transpose(tp, h2[:, c * P:(c + 1) * P], g.ident)
            nc.vector.tensor_copy(out=g.h2Tb[c][:, trows], in_=tp)
            dst = g.h2T2[c // 2][:, (c % 2) * TQ + ts * P:(c % 2) * TQ + (ts + 1) * P]
            nc.vector.tensor_scalar(
                out=dst, in0=tp, scalar1=S_H, scalar2=None, op0=ALU.mult)


def _emit_up_unit(g, unit, uwtp, upps):
    """One up-projection unit for the first token half (512 tokens).

    unit = ('f8', f) for an fp8 chunk or ('fb', fb) for a bf16 chunk.
    Writes into the persistent hid tiles g.hid8_h[f // 2] / g.hidb_h[fb]."""
    nc = g.nc
    kind, f = unit
    pu = upps.tile([P, 512], f32, name="pu")
    if kind == 'f8':
        for c2 in range(NE2):
            h2v = g.h2T2[c2].rearrange("p (k t) -> p k t", k=2)
            nc.tensor.matmul(
                pu, g.uw8_sb[c2][:, :, f * P:(f + 1) * P], h2v[:, :, 0:512],
                start=(c2 == 0), stop=(c2 == NE2 - 1), perf_mode=DR,
            )
        hid8v = g.hid8_h[f // 2].rearrange("p (k t) -> p k t", k=2)
        if g.has_ub:
            nc.scalar.activation(out=hid8v[:, f % 2, :], in_=pu, func=AF.Relu,
                                 bias=g.ub_sb[:, f:f + 1], scale=g.c_u)
        else:
            # relu on DVE (ACT is saturated by softmax exps in this window)
            nc.vector.tensor_scalar(out=hid8v[:, f % 2, :], in0=pu,
                                    scalar1=0.0, scalar2=g.c_u,
                                    op0=ALU.max, op1=ALU.mult)
    else:
        uwt = uwtp.tile([P, NE * P], bf16, name="uwt")
        nc.sync.dma_start(out=uwt, in_=g.uwb_d[f])
        for c in range(NE):
            nc.tensor.matmul(
                pu, uwt[:, c * P:(c + 1) * P], g.h2Tb[c][:, 0:512],
                start=(c == 0), stop=(c == NE - 1),
            )
        if g.has_ub:
            nc.scalar.activation(out=g.hidb_h[f], in_=pu, func=AF.Relu,
                                 bias=g.ub_sb[:, NF8 + f:NF8 + f + 1],
                                 scale=S_HID)
        else:
            nc.vector.tensor_scalar(out=g.hidb_h[f], in0=pu,
                                    scalar1=0.0, scalar2=S_HID,
                                    op0=ALU.max, op1=ALU.mult)


def _emit_down_half(g, dwp8, dwpb, outp, dnps):
    """Down-projection for the first token half from the staged hid tiles."""
    nc = g.nc
    for grp in range(2):  # 256-token groups within the half
        gcols = slice(grp * 256, (grp + 1) * 256)
        dn = [dnps.tile([P, E], f32, name=f"dn{j}") for j in range(2)]
        for f2 in range(NF82):
            dwt = dwp8.tile([P, 2 * E], fp8, name="dwt8")
            nc.sync.dma_start(out=dwt, in_=g.dw8_d[f2])
            dwv = dwt.rearrange("p (k e) -> p k e", k=2)
            hid8v = g.hid8_h[f2].rearrange("p (k t) -> p k t", k=2)[:, :, gcols]
            for t2 in range(2):
                for j in range(2):
                    nc.tensor.matmul(
                        dn[t2][:, j * 512:(j + 1) * 512],
                        hid8v[:, :, t2 * P:(t2 + 1) * P],
                        dwv[:, :, j * 512:(j + 1) * 512],
                        start=(f2 == 0), stop=False, perf_mode=DR,
                    )
        for fb in range(NFB):
            dwt = dwpb.tile([P, E], bf16, name="dwtb")
            nc.sync.dma_start(out=dwt, in_=g.dwb_d[fb])
            hidv = g.hidb_h[fb][:, gcols]
            for t2 in range(2):
                for j in range(2):
                    nc.tensor.matmul(
                        dn[t2][:, j * 512:(j + 1) * 512],
                        hidv[:, t2 * P:(t2 + 1) * P],
                        dwt[:, j * 512:(j + 1) * 512],
                        start=False, stop=(fb == NFB - 1),
                    )
        for t2 in range(2):
            ti = grp * 2 + t2
            ot = outp.tile([P, E], f32, name="ot")
            nc.vector.tensor_scalar(out=ot, in0=dn[t2], scalar1=g.c_d,
                                    scalar2=None, op0=ALU.mult)
            if g.has_db:
                nc.gpsimd.tensor_add(out=ot, in0=ot, in1=g.db_bc)
            nc.gpsimd.tensor_add(out=ot, in0=ot, in1=g.x2_tiles[ti])
            nc.sync.dma_start(out=g.out_d[ti * P:(ti + 1) * P, :], in_=ot)


def _emit_mlp_fused(g, q, hidp8, hidpb, uwtp, dwp8, dwpb, outp, upps, dnps):
    """Fused up+down for one 256-token quarter (second token half)."""
    nc = g.nc
    TQQ = 256
    qcols = slice(q * TQQ, (q + 1) * TQQ)
    dn = [dnps.tile([P, E], f32, name=f"dn{j}") for j in range(2)]
    for f2 in range(NF82):
        hid8 = hidp8.tile([P, 2 * TQQ], fp8, name="hid8")
        hid8v = hid8.rearrange("p (k t) -> p k t", k=2)
        for sub in range(2):
            f = 2 * f2 + sub
            pu = upps.tile([P, TQQ], f32, name="puq")
            for c2 in range(NE2):
                h2v = g.h2T2[c2].rearrange("p (k t) -> p k t", k=2)
                nc.tensor.matmul(
                    pu, g.uw8_sb[c2][:, :, f * P:(f + 1) * P],
                    h2v[:, :, qcols],
                    start=(c2 == 0), stop=(c2 == NE2 - 1), perf_mode=DR,
                )
            nc.scalar.activation(out=hid8v[:, sub, :], in_=pu, func=AF.Relu,
                                 bias=g.ub_sb[:, f:f + 1], scale=g.c_u)
        dwt = dwp8.tile([P, 2 * E], fp8, name="dwt8")
        nc.sync.dma_start(out=dwt, in_=g.dw8_d[f2])
        dwv = dwt.rearrange("p (k e) -> p k e", k=2)
        for t2 in range(2):
            for j in range(2):
                nc.tensor.matmul(
                    dn[t2][:, j * 512:(j + 1) * 512],
                    hid8v[:, :, t2 * P:(t2 + 1) * P],
                    dwv[:, :, j * 512:(j + 1) * 512],
                    start=(f2 == 0), stop=False, perf_mode=DR,
                )
    for fb in range(NFB):
        uwt = uwtp.tile([P, NE * P], bf16, name="uwt")
        nc.sync.dma_start(out=uwt, in_=g.uwb_d[fb])
        pu = upps.tile([P, TQQ], f32, name="puq")
        for c in range(NE):
            nc.tensor.matmul(
                pu, uwt[:, c * P:(c + 1) * P], g.h2Tb[c][:, qcols],
                start=(c == 0), stop=(c == NE - 1),
            )
        hid = hidpb.tile([P, TQQ], bf16, name="hidb")
        nc.scalar.activation(out=hid, in_=pu, func=AF.Relu,
                             bias=g.ub_sb[:, NF8 + fb:NF8 + fb + 1],
                             scale=S_HID)
        dwt = dwpb.tile([P, E], bf16, name="dwtb")
        nc.sync.dma_start(out=dwt, in_=g.dwb_d[fb])
        for t2 in range(2):
            for j in range(2):
                nc.tensor.matmul(
                    dn[t2][:, j * 512:(j + 1) * 512],
                    hid[:, t2 * P:(t2 + 1) * P],
                    dwt[:, j * 512:(j + 1) * 512],
                    start=False, stop=(fb == NFB - 1),
                )
    for t2 in range(2):
        ti = q * 2 + t2
        ot = outp.tile([P, E], f32, name="ot")
        nc.vector.tensor_scalar(out=ot, in0=dn[t2], scalar1=g.c_d,
                                scalar2=None, op0=ALU.mult)
        if g.has_db:
            nc.gpsimd.tensor_add(out=ot, in0=ot, in1=g.db_bc)
        nc.gpsimd.tensor_add(out=ot, in0=ot, in1=g.x2_tiles[ti])
        nc.sync.dma_start(out=g.out_d[ti * P:(ti + 1) * P, :], in_=ot)


def _build(flags, sexp):
    has_qb, has_pb, has_db, has_ub = flags
    e_wq, e_wk, e_wv, e_pw, e_uw, e_dw = sexp
    s_wq, s_wk = 2.0 ** e_wq, 2.0 ** e_wk
    s_wv, s_pw = 2.0 ** e_wv, 2.0 ** e_pw
    s_uw, s_dw = 2.0 ** e_uw, 2.0 ** e_dw
    nc = bacc.Bacc("TRN2", target_bir_lowering=False, debug=False, num_devices=8)

    g = _Ctx()
    g.nc = nc
    g.has_qb, g.has_pb, g.has_db, g.has_ub = flags
    g.c_s = 1.0 / (S_K * S_H * s_wq)          # exp input dequant
    g.c_k = S_K / (S_H * s_wk)                # K psum -> fp8 kt
    g.c_va = S_VA / (S_H * s_wv)              # V psum -> fp8 va
    g.se_sc = S_CAT / S_VA                    # folded into 1/denom
    g.c_p = 1.0 / (S_CAT * s_pw)              # proj psum dequant
    g.c_u = S_HID / (S_H * s_uw)              # up psum -> relu input scale
    g.c_d = 1.0 / (S_HID * s_dw)              # down psum dequant
    g.xkv_d = nc.dram_tensor("xkv", [T, E], f32, kind="ExternalInput")
    g.wq_d = nc.dram_tensor("wq", [NE2, NPAIR, P, 2 * P], fp8,
                            kind="ExternalInput")
    g.wk_d = nc.dram_tensor("wk", [NE2, NPAIR, P, 2 * P], fp8,
                            kind="ExternalInput")
    g.wv_d = nc.dram_tensor("wv", [NE2, P, 2 * E], fp8, kind="ExternalInput")
    g.vrow_d = nc.dram_tensor("vrow", [1, 2 * H * VW], fp8,
                              kind="ExternalInput")
    g.pw_d = nc.dram_tensor("pw", [NE2, P, 2 * E], fp8, kind="ExternalInput")
    g.uw8_d = nc.dram_tensor("uw8", [NE2, P, 2 * NF8 * P], fp8,
                             kind="ExternalInput")
    g.uwb_d = nc.dram_tensor("uwb", [NFB, P, NE * P], bf16,
                             kind="ExternalInput")
    g.ub_d = nc.dram_tensor("ub", [P, NF], f32, kind="ExternalInput")
    g.dw8_d = nc.dram_tensor("dw8", [NF82, P, 2 * E], fp8,
                             kind="ExternalInput")
    g.dwb_d = nc.dram_tensor("dwb", [NFB, P, E], bf16, kind="ExternalInput")
    if has_qb:
        g.qb_d = nc.dram_tensor("qb", [P, NPAIR], f32, kind="ExternalInput")
        g.kb_d = nc.dram_tensor("kb", [P, NPAIR], f32, kind="ExternalInput")
        g.vbrow_d = nc.dram_tensor("vbrow", [1, E], bf16, kind="ExternalInput")
    if has_pb:
        g.pbrow_d = nc.dram_tensor("pbrow", [1, E], f32, kind="ExternalInput")
    if has_db:
        g.dbrow_d = nc.dram_tensor("dbrow", [1, E], f32, kind="ExternalInput")
    g.out_d = nc.dram_tensor("out", [TQ, E], f32, kind="ExternalOutput")

    with tile.TileContext(nc) as tc:
        with (
            tc.tile_pool(name="consts", bufs=1) as consts,
            tc.tile_pool(name="stat", bufs=4) as stat,
            tc.tile_pool(name="catp", bufs=1) as catp,
            tc.tile_pool(name="x2p", bufs=1) as x2p,
            tc.tile_pool(name="h2Tp", bufs=1) as h2Tp,
        ):
            g.consts, g.stat = consts, stat
            _emit_consts(g)
            _emit_all(g, tc, catp, x2p, h2Tp)

    nc.finalize()
    return nc


def _emit_all(g, tc, catp, x2p, h2Tp):
    g.catT2 = [catp.tile([P, 2 * TQ], fp8, name=f"catT{c2}")
               for c2 in range(NE2)]
    g.x2_tiles = [x2p.tile([P, E], f32, name=f"x2_{i}") for i in range(NTS)]
    g.h2T2 = [h2Tp.tile([P, 2 * TQ], fp8, name=f"h2T8{c2}")
              for c2 in range(NE2)]
    g.h2Tb = [h2Tp.tile([P, TQ], bf16, name=f"h2Tb{c}") for c in range(NE)]

    g.pwp = tc.alloc_tile_pool(name="pwp", bufs=1)
    g.uwp = tc.alloc_tile_pool(name="uwp", bufs=1)
    g.hidhp = tc.alloc_tile_pool(name="hidh", bufs=1)
    g.hid8_h = [g.hidhp.tile([P, 2 * 512], fp8, name=f"h8h{f2}")
                for f2 in range(NF82)]
    g.hidb_h = [g.hidhp.tile([P, 512], bf16, name=f"hbh{fb}")
                for fb in range(NFB)]

    with (
        tc.tile_pool(name="vaug", bufs=1) as vap,
        tc.tile_pool(name="qtp", bufs=1) as qtp,
        tc.tile_pool(name="ktp", bufs=1) as ktp,
    ):
        g.va2 = [vap.tile([P, 2 * H * VW], fp8, name=f"va{s2}")
                 for s2 in range(NST2)]
        qts = [qtp.tile([P, TQ], bf16, name=f"qt{p}") for p in range(NPAIR)]
        kts = [ktp.tile([P, T], fp8, name=f"kt{p}") for p in range(NPAIR)]

        with (
            tc.tile_pool(name="hp", bufs=4) as hp,
            tc.tile_pool(name="hTp", bufs=1) as hTp,
        ):
            g.hp = hp
            g.hT2 = [hTp.tile([P, 2 * T], fp8, name=f"hT{c2}")
                     for c2 in range(NE2)]
            with (
                tc.tile_pool(name="xk", bufs=3) as xkp,
                tc.tile_pool(name="tps", bufs=2, space="PSUM") as tps,
            ):
                _emit_ln1_transpose(g, xkp, tps)
            with (
                tc.tile_pool(name="wvp", bufs=1) as wvp,
                tc.tile_pool(name="vps", bufs=4, space="PSUM") as vps,
            ):
                _emit_v(g, wvp, vps)
            with (
                tc.tile_pool(name="wqk", bufs=10) as wqkp,
                tc.tile_pool(name="qkps", bufs=2, space="PSUM") as qkps,
            ):
                for p in range(NPAIR):
                    _emit_qkt_pair(g, p, qts[p], kts[p], wqkp, qkps)
        # hT2 freed here (16KB back) before the attention phase

        with (
            tc.tile_pool(name="ptp", bufs=4) as ptp,
            tc.tile_pool(name="smp", bufs=2) as smp,
            tc.tile_pool(name="atps", bufs=1, space="PSUM") as atps,
            tc.tile_pool(name="scps", bufs=1, space="PSUM") as scps,
        ):
            # query-half 0 attention (no MLP work ready to overlap yet)
            for p in range(NPAIR):
                _emit_attn_pair(g, p, 0, qts[p], kts[p], ptp, smp, scps, atps)
                if p == 0:
                    # prefetch proj/up weights on the idle SWDGE queue
                    g.pw_sb = []
                    for c2 in range(NE2):
                        w = g.pwp.tile([P, 2 * E], fp8, name=f"pw{c2}")
                        g.nc.gpsimd.dma_start(out=w, in_=g.pw_d[c2])
                        g.pw_sb.append(w)
                if p == 1:
                    g.uw8_sb = []
                    for c2 in range(NE2):
                        w = g.uwp.tile([P, 2 * NF8 * P], fp8, name=f"uw8{c2}")
                        g.nc.gpsimd.dma_start(out=w, in_=g.uw8_d[c2])
                        g.uw8_sb.append(w.rearrange("p (k m) -> p k m", k=2))
            # query-half 1 attention overlapped with proj/LN2 + up of half 0
            with (
                tc.tile_pool(name="xq2", bufs=3) as xq2p,
                tc.tile_pool(name="h2p", bufs=3) as h2p,
            ):
                with tc.tile_pool(name="pps", bufs=1, space="PSUM") as pps:
                    proj_units = [
                        (lambda t=ts: _emit_proj_ts(g, t, xq2p, h2p, pps,
                                                    pps))
                        for ts in range(0, 4)
                    ]
                    _emit_attn_pair(g, 0, 1, qts[0], kts[0], ptp, smp, scps,
                                    atps, fillers=proj_units)
                    _emit_attn_pair(g, 1, 1, qts[1], kts[1], ptp, smp, scps,
                                    atps, fillers=proj_units)
                    while proj_units:
                        proj_units.pop(0)()
                with (
                    tc.tile_pool(name="uwtp", bufs=3) as uwtp,
                    tc.tile_pool(name="upps", bufs=2, space="PSUM") as upps,
                ):
                    up_units = [
                        (lambda u=unit: _emit_up_unit(g, u, uwtp, upps))
                        for unit in ([('f8', f) for f in range(NF8)]
                                     + [('fb', fb) for fb in range(NFB)])
                    ]
                    for p in range(2, NPAIR):
                        _emit_attn_pair(g, p, 1, qts[p], kts[p], ptp, smp,
                                        scps, atps, fillers=up_units)
                    while up_units:
                        up_units.pop(0)()

    # tail: proj/LN2 of half 1, down of half 0, fused MLP of half 1
    with (
        tc.tile_pool(name="xq2", bufs=3) as xq2p,
        tc.tile_pool(name="h2p", bufs=3) as h2p,
        tc.tile_pool(name="hidp8", bufs=3) as hidp8,
        tc.tile_pool(name="hidpb", bufs=6) as hidpb,
        tc.tile_pool(name="uwtp", bufs=3) as uwtp,
        tc.tile_pool(name="dwp8", bufs=2) as dwp8,
        tc.tile_pool(name="dwpb", bufs=4) as dwpb,
        tc.tile_pool(name="outp", bufs=3) as outp,
        tc.tile_pool(name="pps", bufs=1, space="PSUM") as pps,
        tc.tile_pool(name="upps", bufs=2, space="PSUM") as upps,
        tc.tile_pool(name="dnps", bufs=1, space="PSUM") as dnps,
    ):
        _emit_proj_ln2(g, range(4, 8), xq2p, h2p, pps, pps)
        _emit_down_half(g, dwp8, dwpb, outp, dnps)
        _emit_mlp_fused(g, 2, hidp8, hidpb, uwtp, dwp8, dwpb, outp, upps,
                        dnps)
        _emit_mlp_fused(g, 3, hidp8, hidpb, uwtp, dwp8, dwpb, outp, upps,
                        dnps)
    g.hidhp.release()
    g.uwp.release()
    g.pwp.release()


def _get_nc(flags, sexp):
    key = (flags, sexp)
    if key not in _BUILD_CACHE:
        _BUILD_CACHE[key] = _build(flags, sexp)
    return _BUILD_CACHE[key]


def _po2_exp(w):
    """Power-of-2 exponent e such that |w|*2^e lands in (112, 224]."""
    amax = float(np.abs(w).max())
    if amax == 0.0:
        return 0
    return int(np.floor(np.log2(224.0 / amax)))


def _prep(x, Wq, Wk, Wv, proj_w, proj_b, ln1_g, ln1_b, ln2_g, ln2_b,
          up_w, up_b, down_w, down_b):
    """Host-side shard + weight fold/quantize/layout.

    Returns (flags, sexp, in_maps)."""
    bfl = ml_dtypes.bfloat16
    f8l = ml_dtypes.float8_e4m3
    x = np.ascontiguousarray(np.asarray(x, dtype=np.float32))
    Wq = np.asarray(Wq, np.float32)
    Wk = np.asarray(Wk, np.float32)
    Wv = np.asarray(Wv, np.float32)
    g1 = np.asarray(ln1_g, np.float32)
    b1 = np.asarray(ln1_b, np.float32)
    g2 = np.asarray(ln2_g, np.float32)
    b2 = np.asarray(ln2_b, np.float32)
    proj_w = np.asarray(proj_w, np.float32)
    up_w = np.asarray(up_w, np.float32)
    down_w = np.asarray(down_w, np.float32)

    # [H, E, D] -> [E, H*D]; fold attention scale into Q, LN1 gain into all
    wq_all = (Wq * (D ** -0.5)).transpose(1, 0, 2).reshape(E, E)
    wk_all = Wk.transpose(1, 0, 2).reshape(E, E)
    wv_all = Wv.transpose(1, 0, 2).reshape(E, E)
    qb_vec = b1 @ wq_all
    kb_vec = b1 @ wk_all
    vb_vec = b1 @ wv_all
    wq_f = g1[:, None] * wq_all
    wk_f = g1[:, None] * wk_all
    wv_f = g1[:, None] * wv_all
    uw_f = g2[:, None] * up_w
    ub_f = np.asarray(up_b, np.float32) + b2 @ up_w

    e_wq, e_wk, e_wv = _po2_exp(wq_f), _po2_exp(wk_f), _po2_exp(wv_f)
    e_pw, e_uw, e_dw = _po2_exp(proj_w), _po2_exp(uw_f), _po2_exp(down_w)
    sexp = (e_wq, e_wk, e_wv, e_pw, e_uw, e_dw)

    def _qkpair_chunks(w, e):  # [E, E] -> [NE2, NPAIR, P, 2*P] fp8
        ws = (w * 2.0 ** e).reshape(NE2, 2, P, NPAIR, P)
        return np.ascontiguousarray(
            ws.transpose(0, 3, 2, 1, 4).reshape(NE2, NPAIR, P, 2 * P)
            .astype(f8l))

    def _kpair(w, e, ncols):  # [E, ncols] -> [NE2, P, 2*ncols] fp8
        ws = (w * 2.0 ** e).reshape(NE2, 2, P, ncols)
        return np.ascontiguousarray(
            ws.transpose(0, 2, 1, 3).reshape(NE2, P, 2 * ncols).astype(f8l))

    vrow = np.zeros((1, 2 * H * VW), np.float32)
    vrow.reshape(2, H, VW)[:, :, D] = 1.0

    # down: first NF8 chunks fp8 (k-pairs), rest bf16 pre-scaled by s_dw
    dw_s = down_w * 2.0 ** e_dw
    dw8 = np.ascontiguousarray(
        dw_s[:NF8 * P].reshape(NF82, 2, P, E).transpose(0, 2, 1, 3)
        .reshape(NF82, P, 2 * E).astype(f8l))
    dwb = np.ascontiguousarray(
        dw_s[NF8 * P:].reshape(NFB, P, E).astype(bfl))
    uw8 = _kpair(uw_f[:, :NF8 * P], e_uw, NF8 * P)
    # bf16 up chunks: [NFB, P(e within chunk), NE*P] so one DMA per f-chunk
    uwb = np.ascontiguousarray(
        uw_f[:, NF8 * P:].reshape(NE, P, NFB, P).transpose(2, 1, 0, 3)
        .reshape(NFB, P, NE * P).astype(bfl))

    has_qb = bool(np.any(b1 != 0))
    has_pb = bool(np.any(np.asarray(proj_b) != 0))
    has_db = bool(np.any(np.asarray(down_b) != 0))
    has_ub = bool(np.any(ub_f != 0))
    flags = (has_qb, has_pb, has_db, has_ub)

    shared = {
        "wq": _qkpair_chunks(wq_f, e_wq),
        "wk": _qkpair_chunks(wk_f, e_wk),
        "wv": _kpair(wv_f, e_wv, E),
        "vrow": vrow.astype(f8l),
        "pw": _kpair(proj_w, e_pw, E),
        "uw8": uw8,
        "uwb": uwb,
        "ub": np.ascontiguousarray(
            (S_HID * ub_f).reshape(NF, P).T.astype(np.float32)),
        "dw8": dw8,
        "dwb": dwb,
    }
    if has_qb:
        shared["qb"] = np.ascontiguousarray(
            (qb_vec * (S_H * 2.0 ** e_wq)).reshape(NPAIR, P).T
            .astype(np.float32))
        shared["kb"] = np.ascontiguousarray(
            (kb_vec * (S_H * 2.0 ** e_wk)).reshape(NPAIR, P).T
            .astype(np.float32))
        shared["vbrow"] = (vb_vec * (S_H * 2.0 ** e_wv)).reshape(1, E)\
            .astype(bfl)
    if has_pb:
        shared["pbrow"] = np.asarray(proj_b, np.float32).reshape(1, E)
    if has_db:
        shared["dbrow"] = np.asarray(down_b, np.float32).reshape(1, E)

    in_maps = []
    for c in range(8):
        b, half = c // 2, c % 2
        xb = x[b]
        if half == 1:
            xb = np.concatenate([xb[TQ:], xb[:TQ]], axis=0)
        in_maps.append({"xkv": np.ascontiguousarray(xb), **shared})
    return flags, sexp, in_maps


def kernel(**inputs) -> np.ndarray:
    flags, sexp, in_maps = _prep(**inputs)
    nc = _get_nc(flags, sexp)
    res = run_bass_kernel_spmd(nc, in_maps, core_ids=list(range(8)))
    out = np.empty((B, T, E), np.float32)
    for c in range(8):
        b, half = c // 2, c % 2
        out[b, half * TQ:(half + 1) * TQ, :] = res.results[c]["out"]
    return out
